# revision 1
# baseline (speedup 1.0000x reference)
"""GCN message-passing kernel for 8 Trainium2 NeuronCores.

Model (PyG GCNConv x3 + MLP head + softmax):
    A01 = adjacency + self loops (unit weights), deg = in-degree over A01
    conv(H, W) = D^-1/2 A01 D^-1/2 (H @ W)
    h = x; h = leaky(conv(h, Wg_l)) x3
    y = softmax(leaky(leaky(h @ Wfc1) @ Wfc2))

Key algebraic rewrite: leaky_relu is positively homogeneous, so the
D^-1/2 factors can be pulled out of every leaky() and folded into the
per-layer "message table" G_l:
    G_1 = D^-1/2 (x @ Wg0)
    Ht_{l+1} = leaky(A01 @ G_l)            (pure 0/1 segment-sum!)
    G_{l+1} = D^-1 (Ht_{l+1} @ Wg_l)
    final: z = D^-1/2 leaky(leaky(Ht_4 @ Wfc1) @ Wfc2), out = softmax(z)

Sharding: destination nodes are split into 8 contiguous blocks of 6250
(padded to 6272 = 49 windows of 128). Each layer: every core computes its
G shard (matmul + per-row scale), an AllGather builds the full G table in
DRAM, then each core gathers source rows for its edges (dma_gather,
int16 indices, table split in two <32768-row halves), builds a 0/1
one-hot matrix per 128-edge chunk on the Vector engine (is_equal vs an
iota row), and accumulates  msg^T @ onehot  into a PSUM window on the
TensorEngine.  The flush produces the next layer's activations already
transposed (feat x rows), which is exactly the lhsT layout the next
matmul needs.
"""

import numpy as np

P = 128
N_CORES = 8


def _gw(NW):
    """Windows per gather group."""
    return 7 if NW % 7 == 0 else 1


# --------------------------------------------------------------------------
# Host-side preprocessing: shard edges by destination, pad to fixed chunk
# counts (SPMD requires an identical instruction stream on all cores).
# --------------------------------------------------------------------------
def _preprocess(x, edge_index):
    N, D = x.shape
    assert D == P
    NL = N // N_CORES                      # real nodes per core
    NW = (NL + P - 1) // P                 # windows per core
    NLP = NW * P                           # padded nodes per core
    NGP = N_CORES * NLP                    # padded global nodes
    HALF = 32768                           # int16 gather index limit

    src = np.asarray(edge_index[0], dtype=np.int64)
    dst = np.asarray(edge_index[1], dtype=np.int64)
    loop = np.arange(N, dtype=np.int64)
    src_all = np.concatenate([src, loop])
    dst_all = np.concatenate([dst, loop])

    # in-degree (counts, incl self loops) -- pure index metadata
    deg = np.bincount(dst_all, minlength=N).astype(np.float32)

    # padded global id of each source node
    sowner = src_all // NL
    spid = sowner * NLP + (src_all - sowner * NL)

    owner = dst_all // NL                  # destination owner core
    lid = dst_all - owner * NL             # local dest id on that core
    w = lid // P                           # window
    dr = (lid % P).astype(np.float32)      # one-hot row within window
    half = (spid >= HALF).astype(np.int64)
    srel = np.where(half == 1, spid - HALF, spid)  # idx within its half

    # bucket key: (core, window, half)
    key = (owner * NW + w) * 2 + half
    nbuckets = N_CORES * NW * 2
    order = np.argsort(key, kind="stable")
    key_s = key[order]
    srel_s = srel[order]
    dr_s = dr[order]

    counts = np.bincount(key_s, minlength=nbuckets)
    clo = counts.reshape(-1, 2)[:, 0]
    chi = counts.reshape(-1, 2)[:, 1]
    NCHL = int(np.ceil(clo.max() / P))     # lo chunks per window
    NCHH = int(np.ceil(chi.max() / P))     # hi chunks per window
    CAPL, CAPH = NCHL * P, NCHH * P

    # destination slot of each edge inside the padded per-bucket arrays
    cap = np.where((np.arange(nbuckets) % 2) == 0, CAPL, CAPH)
    base = np.zeros(nbuckets + 1, dtype=np.int64)
    np.cumsum(cap, out=base[1:])
    start = np.zeros(nbuckets, dtype=np.int64)
    start[1:] = np.cumsum(counts)[:-1]
    within = np.arange(len(key_s)) - start[key_s]
    dest = base[key_s] + within

    total_cap = int(base[-1])
    idx_flat = np.zeros(total_cap, dtype=np.int16)
    dr_flat = np.full(total_cap, 200.0, dtype=np.float32)
    idx_flat[dest] = srel_s.astype(np.int16)
    dr_flat[dest] = dr_s

    # per-core views: [NW, 2-half blocks]
    per_core = []
    cap_core = NW * (CAPL + CAPH)
    for c in range(N_CORES):
        seg_i = idx_flat[c * cap_core:(c + 1) * cap_core]
        seg_d = dr_flat[c * cap_core:(c + 1) * cap_core]
        # window-major [NW, CAPL+CAPH]
        seg_i = seg_i.reshape(NW, CAPL + CAPH)
        seg_d = seg_d.reshape(NW, CAPL + CAPH)
        ilo = seg_i[:, :CAPL]              # [NW, CAPL]
        ihi = seg_i[:, CAPL:]
        dlo = seg_d[:, :CAPL]
        dhi = seg_d[:, CAPL:]
        per_core.append((ilo, ihi, dlo, dhi))

    meta = dict(N=N, NL=NL, NW=NW, NLP=NLP, NGP=NGP, HALF=HALF,
                NCHL=NCHL, NCHH=NCHH, deg=deg)
    return per_core, meta


def _wrap_idx_groups(idx_win, ngrp, gw):
    """idx_win: [NW, CAP] int16, window-major edge slots.
    Returns [128, NW*CAP/16] int16 in dma_gather's wrapped layout:
    per gather call (= group of gw windows) logical index j lives at
    [j % 16, j // 16], replicated 8x across the 128 partitions."""
    NW, CAP = idx_win.shape
    cols = []
    for g in range(ngrp):
        block = idx_win[g * gw:(g + 1) * gw].reshape(-1)   # [gw*CAP]
        m = block.reshape(-1, 16).T                        # [16, gw*CAP/16]
        cols.append(np.tile(m, (8, 1)))                    # [128, ...]
    return np.ascontiguousarray(np.concatenate(cols, axis=1))


def _build_core_inputs(x, Ws, per_core, meta):
    """Build the per-core device input dict."""
    N, NL, NW, NLP = meta["N"], meta["NL"], meta["NW"], meta["NLP"]
    NCHL, NCHH = meta["NCHL"], meta["NCHH"]
    deg = meta["deg"]
    GW = _gw(NW)
    ngrp = NW // GW
    Wg0, Wg1, Wg2, Wfc1, Wfc2 = Ws

    iota = np.tile(np.arange(P, dtype=np.float32), (P, 1))
    # Wfc2 [256, 2] -> [128, 4]: cols 0:2 first half of u, 2:4 second half
    Wfc2p = np.concatenate([Wfc2[:P, :], Wfc2[P:, :]], axis=1)
    Wfc2p = np.ascontiguousarray(Wfc2p, dtype=np.float32)

    in_maps = []
    for c in range(N_CORES):
        ilo, ihi, dlo, dhi = per_core[c]
        xs = np.zeros((NLP, P), dtype=np.float32)
        xs[:NL] = x[c * NL:(c + 1) * NL]
        x_t = np.ascontiguousarray(xs.T)                   # [128, NLP]

        degp = np.ones(NLP, dtype=np.float32)
        degp[:NL] = deg[c * NL:(c + 1) * NL]
        deg_t = np.ascontiguousarray(degp.reshape(NW, P).T)  # [128, NW]

        # dstrel: [128, NW*NCH] col = w*NCH + k, row p = edge slot
        drl = np.ascontiguousarray(
            dlo.reshape(NW, NCHL, P).transpose(2, 0, 1).reshape(P, NW * NCHL))
        drh = np.ascontiguousarray(
            dhi.reshape(NW, NCHH, P).transpose(2, 0, 1).reshape(P, NW * NCHH))

        hi_part = {}
        if NCHH:
            hi_part = {"idx_hi": _wrap_idx_groups(ihi, ngrp, GW),
                       "dstrel_hi": drh}
        in_maps.append({
            "x_t": x_t,
            "deg_t": deg_t,
            "idx_lo": _wrap_idx_groups(ilo, ngrp, GW),
            "dstrel_lo": drl,
            **hi_part,
            "iota": iota,
            "Wg0": np.ascontiguousarray(Wg0, dtype=np.float32),
            "Wg1": np.ascontiguousarray(Wg1, dtype=np.float32),
            "Wg2": np.ascontiguousarray(Wg2, dtype=np.float32),
            "Wfc1": np.ascontiguousarray(Wfc1, dtype=np.float32),
            "Wfc2p": Wfc2p,
        })
    return in_maps


# --------------------------------------------------------------------------
# Device program
# --------------------------------------------------------------------------
def _build_bass(meta, mock_cc=False, opts=None):
    opts = opts or {}
    from concourse import bass, bacc, mybir
    import concourse.tile as tile

    NW, NLP, NGP, HALF = meta["NW"], meta["NLP"], meta["NGP"], meta["HALF"]
    NCHL, NCHH = meta["NCHL"], meta["NCHH"]
    GW = _gw(NW)
    NGRP = NW // GW
    GLL = GW * NCHL * P                    # lo idxs per gather call
    GLH = GW * NCHH * P
    f32 = mybir.dt.float32
    bf16 = mybir.dt.bfloat16
    i16 = mybir.dt.int16
    ALL = [list(range(N_CORES))]

    nc = bacc.Bacc("TRN2", target_bir_lowering=False, debug=False,
                   num_devices=N_CORES)

    x_t_d = nc.dram_tensor("x_t", [P, NLP], f32, kind="ExternalInput")
    deg_d = nc.dram_tensor("deg_t", [P, NW], f32, kind="ExternalInput")
    ilo_d = nc.dram_tensor("idx_lo", [P, NW * NCHL * 8], i16, kind="ExternalInput")
    drl_d = nc.dram_tensor("dstrel_lo", [P, NW * NCHL], f32, kind="ExternalInput")
    if NCHH:
        ihi_d = nc.dram_tensor("idx_hi", [P, NW * NCHH * 8], i16,
                               kind="ExternalInput")
        drh_d = nc.dram_tensor("dstrel_hi", [P, NW * NCHH], f32,
                               kind="ExternalInput")
    iota_d = nc.dram_tensor("iota", [P, P], f32, kind="ExternalInput")
    wg_d = [nc.dram_tensor(f"Wg{i}", [P, P], f32, kind="ExternalInput")
            for i in range(3)]
    wfc1_d = nc.dram_tensor("Wfc1", [P, 256], f32, kind="ExternalInput")
    wfc2_d = nc.dram_tensor("Wfc2p", [P, 4], f32, kind="ExternalInput")
    out_d = nc.dram_tensor("out", [NLP, 2], f32, kind="ExternalOutput")

    with tile.TileContext(nc) as tc:
        with (
            tc.tile_pool(name="const", bufs=1) as cpool,
            tc.tile_pool(name="msg", bufs=2) as mpool,
            tc.tile_pool(name="oh", bufs=2) as ohpool,
            tc.tile_pool(name="work", bufs=3) as wpool,
            tc.tile_pool(name="acc", bufs=6, space="PSUM") as ppool,
            tc.tile_pool(name="accy", bufs=2, space="PSUM") as p2pool,
            tc.tile_pool(name="dram", bufs=1, space="DRAM") as dpool,
        ):
            # ---- constants / casts ----
            T_a = cpool.tile([P, NLP], bf16, name="T_a")
            nc.gpsimd.dma_start(out=T_a[:], in_=x_t_d[:])   # f32->bf16 cast
            T_b = cpool.tile([P, NLP], bf16, name="T_b")

            iota_sb = cpool.tile([P, P], bf16, name="iota_sb")
            nc.gpsimd.dma_start(out=iota_sb[:], in_=iota_d[:])
            wg_sb = []
            for i in range(3):
                t = cpool.tile([P, P], bf16, name=f"wg_sb{i}")
                nc.gpsimd.dma_start(out=t[:], in_=wg_d[i][:])
                wg_sb.append(t)
            wfc1_sb = cpool.tile([P, 256], bf16, name="wfc1_sb")
            nc.gpsimd.dma_start(out=wfc1_sb[:], in_=wfc1_d[:])
            wfc2_sb = cpool.tile([P, 4], bf16, name="wfc2_sb")
            nc.gpsimd.dma_start(out=wfc2_sb[:], in_=wfc2_d[:])
            drl_sb = cpool.tile([P, NW * NCHL], bf16, name="drl_sb")
            nc.gpsimd.dma_start(out=drl_sb[:], in_=drl_d[:])  # f32->bf16
            ilo_sb = cpool.tile([P, NW * NCHL * 8], i16, name="ilo_sb")
            nc.sync.dma_start(out=ilo_sb[:], in_=ilo_d[:])
            if NCHH:
                drh_sb = cpool.tile([P, NW * NCHH], bf16, name="drh_sb")
                nc.gpsimd.dma_start(out=drh_sb[:], in_=drh_d[:])
                ihi_sb = cpool.tile([P, NW * NCHH * 8], i16, name="ihi_sb")
                nc.sync.dma_start(out=ihi_sb[:], in_=ihi_d[:])

            deg_sb = cpool.tile([P, NW], f32, name="deg_sb")
            nc.sync.dma_start(out=deg_sb[:], in_=deg_d[:])
            invdeg = cpool.tile([P, NW], f32, name="invdeg")
            nc.vector.reciprocal(invdeg[:], deg_sb[:])
            dinv = cpool.tile([P, NW], f32, name="dinv")
            nc.scalar.sqrt(dinv[:], invdeg[:])

            # NB: collective outputs in Local addr space — Shared
            # scratchpad DMA reads measured ~3x slower on the gather path.
            gfull = [dpool.tile([NGP, P], bf16, name=f"gfull{i}")
                     for i in range(3)]
            gloc = [dpool.tile([NLP, P], bf16, name=f"gloc{i}")
                    for i in range(3)]

            Copy = mybir.ActivationFunctionType.Copy

            def g_production(l, Tsrc):
                scale = dinv if l == 0 else invdeg
                for w in range(NW):
                    ps = ppool.tile([P, P], f32, tag="acc", name="psg")
                    nc.tensor.matmul(ps[:], lhsT=Tsrc[:, w * P:(w + 1) * P],
                                     rhs=wg_sb[l][:], start=True, stop=True)
                    gw_t = wpool.tile([P, P], bf16, tag="gw", name="gw_t")
                    nc.scalar.activation(gw_t[:], ps[:], Copy,
                                         bias=0.0, scale=scale[:, w:w + 1])
                    nc.sync.dma_start(out=gloc[l][w * P:(w + 1) * P, :],
                                      in_=gw_t[:])
                if mock_cc:
                    # single-core timing mock: local copy approximating the
                    # AllGather's local write volume
                    for c in range(N_CORES):
                        nc.sync.dma_start(
                            out=gfull[l][c * NLP:(c + 1) * NLP, :],
                            in_=gloc[l][:])
                else:
                    nc.gpsimd.collective_compute(
                        "AllGather", mybir.AluOpType.bypass,
                        replica_groups=ALL,
                        ins=[gloc[l][:]], outs=[gfull[l][:]])

            def leaky_into(dst_ap, ps):
                t = wpool.tile([P, dst_ap.shape[-1]], f32, tag="lk", name="lkt")
                nc.scalar.activation(t[:], ps[:], Copy, bias=0.0, scale=0.01)
                nc.vector.tensor_tensor(out=dst_ap, in0=ps[:], in1=t[:],
                                        op=mybir.AluOpType.max)


            ohb_const = {}
            if opts.get("skip_ohbuild"):
                GWl = _gw(NW)
                KL = GWl * NCHL
                t = cpool.tile([P, KL * P], bf16, name="ohc_lo")
                nc.vector.tensor_tensor(
                    out=t[:].rearrange("p (k r) -> p k r", r=P),
                    in0=iota_sb[:].unsqueeze(1).to_broadcast([P, KL, P]),
                    in1=drl_sb[:, 0:KL].unsqueeze(2)
                        .to_broadcast([P, KL, P]),
                    op=mybir.AluOpType.is_equal)
                ohb_const["lo"] = t
                if NCHH:
                    KH = GWl * NCHH
                    t2 = cpool.tile([P, KH * P], bf16, name="ohc_hi")
                    nc.vector.tensor_tensor(
                        out=t2[:].rearrange("p (k r) -> p k r", r=P),
                        in0=iota_sb[:].unsqueeze(1).to_broadcast([P, KH, P]),
                        in1=drh_sb[:, 0:KH].unsqueeze(2)
                            .to_broadcast([P, KH, P]),
                        op=mybir.AluOpType.is_equal)
                    ohb_const["hi"] = t2

            def scatter(l, Tdst):
                glo_ap = gfull[l][0:min(HALF, NGP), :]
                ghi_ap = gfull[l][HALF:NGP, :] if NCHH else None
                ntot = NCHL + NCHH
                for g in range(NGRP):
                    mlo = mpool.tile([P, GLL], bf16, tag="mlo", name="mlo")
                    if opts.get("fake_gather"):
                        nc.sync.dma_start(
                            out=mlo[:].rearrange("p (c e) -> p c e", e=P),
                            in_=gfull[l][0:GLL, :].rearrange(
                                "(c p) e -> p c e", p=P))
                    else:
                        nc.gpsimd.dma_gather(
                            out_ap=mlo[:].rearrange("p (c e) -> p c e", e=P),
                            in_ap=glo_ap,
                            idxs_ap=ilo_sb[:, g * (GLL // 16):
                                           (g + 1) * (GLL // 16)],
                            num_idxs=GLL, num_idxs_reg=GLL, elem_size=P,
                            single_packet=False)
                    if NCHH:
                        mhi = mpool.tile([P, GLH], bf16, tag="mhi", name="mhi")
                        if opts.get("fake_gather"):
                            nc.sync.dma_start(
                                out=mhi[:].rearrange("p (c e) -> p c e", e=P),
                                in_=gfull[l][0:GLH, :].rearrange(
                                    "(c p) e -> p c e", p=P))
                        else:
                            nc.gpsimd.dma_gather(
                                out_ap=mhi[:].rearrange("p (c e) -> p c e",
                                                        e=P),
                                in_ap=ghi_ap,
                                idxs_ap=ihi_sb[:, g * (GLH // 16):
                                               (g + 1) * (GLH // 16)],
                                num_idxs=GLH, num_idxs_reg=GLH, elem_size=P,
                                single_packet=False)
                    # batched one-hot build: one DVE op per (group, half)
                    KL = GW * NCHL
                    if opts.get("skip_ohbuild"):
                        ohb_lo = ohb_const["lo"]
                    else:
                        ohb_lo = ohpool.tile([P, KL * P], bf16, tag="ohlo",
                                             name="ohb_lo")
                        nc.vector.tensor_tensor(
                            out=ohb_lo[:].rearrange("p (k r) -> p k r", r=P),
                            in0=iota_sb[:].unsqueeze(1)
                                .to_broadcast([P, KL, P]),
                            in1=drl_sb[:, g * KL:(g + 1) * KL].unsqueeze(2)
                                .to_broadcast([P, KL, P]),
                            op=mybir.AluOpType.is_equal)
                    if NCHH:
                        KH = GW * NCHH
                        if opts.get("skip_ohbuild"):
                            ohb_hi = ohb_const["hi"]
                        else:
                            ohb_hi = ohpool.tile([P, KH * P], bf16,
                                                 tag="ohhi", name="ohb_hi",
                                                 bufs=opts.get("hibufs", 1))
                            nc.vector.tensor_tensor(
                                out=ohb_hi[:].rearrange("p (k r) -> p k r",
                                                        r=P),
                                in0=iota_sb[:].unsqueeze(1)
                                    .to_broadcast([P, KH, P]),
                                in1=drh_sb[:, g * KH:(g + 1) * KH]
                                    .unsqueeze(2).to_broadcast([P, KH, P]),
                                op=mybir.AluOpType.is_equal)
                    for wi in range(GW):
                        w = g * GW + wi
                        ps = ppool.tile([P, P], f32, tag="acc", name="pss")
                        if not opts.get("skip_matmul"):
                            for k in range(ntot):
                                if k < NCHL:
                                    c = wi * NCHL + k
                                    m_ap = mlo[:, c * P:(c + 1) * P]
                                    oh_ap = ohb_lo[:, c * P:(c + 1) * P]
                                else:
                                    c = wi * NCHH + (k - NCHL)
                                    m_ap = mhi[:, c * P:(c + 1) * P]
                                    oh_ap = ohb_hi[:, c * P:(c + 1) * P]
                                nc.tensor.matmul(ps[:], lhsT=m_ap, rhs=oh_ap,
                                                 start=(k == 0),
                                                 stop=(k == ntot - 1))
                            if not opts.get("skip_leaky"):
                                leaky_into(Tdst[:, w * P:(w + 1) * P], ps)

            def head(Tsrc):
                X = mybir.AxisListType.X
                Exp = mybir.ActivationFunctionType.Exp
                for w in range(NW):
                    y1t = []
                    for h in range(2):
                        ps1 = ppool.tile([P, P], f32, tag="acc", name="ps1")
                        nc.tensor.matmul(ps1[:],
                                         lhsT=wfc1_sb[:, h * P:(h + 1) * P],
                                         rhs=Tsrc[:, w * P:(w + 1) * P],
                                         start=True, stop=True)
                        yt = wpool.tile([P, P], bf16, tag=f"y1_{h}",
                                        name="yt")
                        leaky_into(yt[:], ps1)
                        y1t.append(yt)
                    ps2 = p2pool.tile([P, 2], f32, tag="y2", name="ps2")
                    nc.tensor.matmul(ps2[:], lhsT=y1t[0][:],
                                     rhs=wfc2_sb[:, 0:2],
                                     start=True, stop=False)
                    nc.tensor.matmul(ps2[:], lhsT=y1t[1][:],
                                     rhs=wfc2_sb[:, 2:4],
                                     start=False, stop=True)
                    y2 = wpool.tile([P, 2], f32, tag="y2s", name="y2")
                    leaky_into(y2[:], ps2)
                    z = wpool.tile([P, 2], f32, tag="z", name="z")
                    nc.scalar.activation(z[:], y2[:], Copy, bias=0.0,
                                         scale=dinv[:, w:w + 1])
                    negm = wpool.tile([P, 1], f32, tag="m", name="negm")
                    nc.vector.reduce_max(out=negm[:], in_=z[:], axis=X,
                                         negate=True)
                    e = wpool.tile([P, 2], f32, tag="e", name="e")
                    nc.scalar.activation(e[:], z[:], Exp,
                                         bias=negm[:, 0:1], scale=1.0)
                    s = wpool.tile([P, 1], f32, tag="s", name="s")
                    nc.vector.reduce_sum(out=s[:], in_=e[:], axis=X)
                    rs = wpool.tile([P, 1], f32, tag="rs", name="rs")
                    nc.vector.reciprocal(rs[:], s[:])
                    o = wpool.tile([P, 2], f32, tag="o", name="o")
                    nc.vector.tensor_scalar(out=o[:], in0=e[:],
                                            scalar1=rs[:, 0:1], scalar2=None,
                                            op0=mybir.AluOpType.mult)
                    nc.sync.dma_start(out=out_d[w * P:(w + 1) * P, :],
                                      in_=o[:])

            if opts.get("skip_scatter"):
                g_production(0, T_a)
                g_production(1, T_a)
                g_production(2, T_a)
                head(T_a)
            else:
                g_production(0, T_a)
                scatter(0, T_b)
                g_production(1, T_b)
                scatter(1, T_a)
                g_production(2, T_a)
                scatter(2, T_b)
                head(T_b)

    nc.compile()
    return nc


# --------------------------------------------------------------------------
# Entry point
# --------------------------------------------------------------------------
LAST_RESULT = None
LAST_NC = None
LAST_IN_MAPS = None
LAST_META = None


def kernel(x, edge_index, Wg0, Wg1, Wg2, Wfc1, Wfc2):
    from concourse.bass_utils import run_bass_kernel_spmd

    global LAST_RESULT, LAST_NC, LAST_IN_MAPS, LAST_META
    x = np.asarray(x)
    edge_index = np.asarray(edge_index)
    per_core, meta = _preprocess(x, edge_index)
    in_maps = _build_core_inputs(
        x, (np.asarray(Wg0), np.asarray(Wg1), np.asarray(Wg2),
            np.asarray(Wfc1), np.asarray(Wfc2)), per_core, meta)
    nc = _build_bass(meta)
    LAST_NC, LAST_IN_MAPS, LAST_META = nc, in_maps, meta
    res = run_bass_kernel_spmd(nc, in_maps, core_ids=list(range(N_CORES)))
    LAST_RESULT = res
    NL = meta["NL"]
    out = np.concatenate([res.results[c]["out"][:NL] for c in range(N_CORES)],
                         axis=0)
    return out.astype(np.float32)



# revision 13
# speedup vs baseline: 7.6341x; 7.6341x over previous
"""GCN message-passing kernel for 8 Trainium2 NeuronCores.

Model (PyG GCNConv x3 + MLP head + softmax):
    A01 = adjacency + self loops (unit weights), deg = in-degree over A01
    conv(H, W) = D^-1/2 A01 D^-1/2 (H @ W)
    h = x; h = leaky(conv(h, Wg_l)) x3
    y = softmax(leaky(leaky(h @ Wfc1) @ Wfc2))

Key algebraic rewrite: leaky_relu is positively homogeneous, so the
D^-1/2 factors can be pulled out of every leaky() and folded into the
per-layer "message table" G_l:
    G_1 = D^-1/2 (x @ Wg0)
    Ht_{l+1} = leaky(A01 @ G_l)            (pure 0/1 segment-sum!)
    G_{l+1} = D^-1 (Ht_{l+1} @ Wg_l)
    final: z = D^-1/2 leaky(leaky(Ht_4 @ Wfc1) @ Wfc2), out = softmax(z)

Sharding: destination nodes are split into 8 contiguous blocks of 6250
(padded to 6272 = 49 windows of 128). Each layer: every core computes its
G shard (matmul + per-row scale), an AllGather builds the full G table in
DRAM, then each core gathers source rows for its edges (dma_gather,
int16 indices, table split in two <32768-row halves), builds a 0/1
one-hot matrix per 128-edge chunk on the Vector engine (is_equal vs an
iota row), and accumulates  msg^T @ onehot  into a PSUM window on the
TensorEngine.  The flush produces the next layer's activations already
transposed (feat x rows), which is exactly the lhsT layout the next
matmul needs.
"""

import numpy as np

P = 128
N_CORES = 8


def _gw(NW):
    """Windows per gather group."""
    return 7 if NW % 7 == 0 else 1


# --------------------------------------------------------------------------
# Host-side preprocessing: shard edges by destination, pad to fixed chunk
# counts (SPMD requires an identical instruction stream on all cores).
# --------------------------------------------------------------------------
def _preprocess(x, edge_index):
    N, D = x.shape
    assert D == P
    NL = N // N_CORES                      # real nodes per core
    NW = (NL + P - 1) // P                 # windows per core
    NLP = NW * P                           # padded nodes per core
    NGP = N_CORES * NLP                    # padded global nodes
    HALF = 32768                           # int16 gather index limit

    src = np.asarray(edge_index[0], dtype=np.int64)
    dst = np.asarray(edge_index[1], dtype=np.int64)
    loop = np.arange(N, dtype=np.int64)
    src_all = np.concatenate([src, loop])
    dst_all = np.concatenate([dst, loop])

    # in-degree (counts, incl self loops) -- pure index metadata
    deg = np.bincount(dst_all, minlength=N).astype(np.float32)

    # padded global id of each source node
    sowner = src_all // NL
    spid = sowner * NLP + (src_all - sowner * NL)

    owner = dst_all // NL                  # destination owner core
    lid = dst_all - owner * NL             # local dest id on that core
    w = lid // P                           # window
    dr = (lid % P).astype(np.float32)      # one-hot row within window
    half = (spid >= HALF).astype(np.int64)
    srel = np.where(half == 1, spid - HALF, spid)  # idx within its half

    # bucket key: (core, window, half)
    key = (owner * NW + w) * 2 + half
    nbuckets = N_CORES * NW * 2
    order = np.argsort(key, kind="stable")
    key_s = key[order]
    srel_s = srel[order]
    dr_s = dr[order]

    counts = np.bincount(key_s, minlength=nbuckets)
    clo = counts.reshape(-1, 2)[:, 0]
    chi = counts.reshape(-1, 2)[:, 1]
    NCHL = int(np.ceil(clo.max() / P))     # lo chunks per window
    NCHH = int(np.ceil(chi.max() / P))     # hi chunks per window
    CAPL, CAPH = NCHL * P, NCHH * P

    # destination slot of each edge inside the padded per-bucket arrays
    cap = np.where((np.arange(nbuckets) % 2) == 0, CAPL, CAPH)
    base = np.zeros(nbuckets + 1, dtype=np.int64)
    np.cumsum(cap, out=base[1:])
    start = np.zeros(nbuckets, dtype=np.int64)
    start[1:] = np.cumsum(counts)[:-1]
    within = np.arange(len(key_s)) - start[key_s]
    dest = base[key_s] + within

    total_cap = int(base[-1])
    idx_flat = np.zeros(total_cap, dtype=np.int16)
    dr_flat = np.full(total_cap, 200.0, dtype=np.float32)
    idx_flat[dest] = srel_s.astype(np.int16)
    dr_flat[dest] = dr_s

    # per-core views: [NW, 2-half blocks]
    per_core = []
    cap_core = NW * (CAPL + CAPH)
    for c in range(N_CORES):
        seg_i = idx_flat[c * cap_core:(c + 1) * cap_core]
        seg_d = dr_flat[c * cap_core:(c + 1) * cap_core]
        # window-major [NW, CAPL+CAPH]
        seg_i = seg_i.reshape(NW, CAPL + CAPH)
        seg_d = seg_d.reshape(NW, CAPL + CAPH)
        ilo = seg_i[:, :CAPL]              # [NW, CAPL]
        ihi = seg_i[:, CAPL:]
        dlo = seg_d[:, :CAPL]
        dhi = seg_d[:, CAPL:]
        per_core.append((ilo, ihi, dlo, dhi))

    meta = dict(N=N, NL=NL, NW=NW, NLP=NLP, NGP=NGP, HALF=HALF,
                NCHL=NCHL, NCHH=NCHH, deg=deg)
    return per_core, meta


def _wrap_idx_groups(idx_win, ngrp, gw):
    """idx_win: [NW, CAP] int16, window-major edge slots.
    Returns [128, NW*CAP/16] int16 in dma_gather's wrapped layout:
    per gather call (= group of gw windows) logical index j lives at
    [j % 16, j // 16], replicated 8x across the 128 partitions."""
    NW, CAP = idx_win.shape
    cols = []
    for g in range(ngrp):
        block = idx_win[g * gw:(g + 1) * gw].reshape(-1)   # [gw*CAP]
        m = block.reshape(-1, 16).T                        # [16, gw*CAP/16]
        cols.append(np.tile(m, (8, 1)))                    # [128, ...]
    return np.ascontiguousarray(np.concatenate(cols, axis=1))


def _build_core_inputs(x, Ws, per_core, meta):
    """Build the per-core device input dict."""
    N, NL, NW, NLP = meta["N"], meta["NL"], meta["NW"], meta["NLP"]
    NCHL, NCHH = meta["NCHL"], meta["NCHH"]
    deg = meta["deg"]
    GW = _gw(NW)
    ngrp = NW // GW
    Wg0, Wg1, Wg2, Wfc1, Wfc2 = Ws

    iota = np.tile(np.arange(P, dtype=np.float32), (P, 1))
    ident = np.eye(P, dtype=np.float32)
    # Wfc2 [256, 2] -> [128, 4]: cols 0:2 first half of u, 2:4 second half
    Wfc2p = np.concatenate([Wfc2[:P, :], Wfc2[P:, :]], axis=1)
    Wfc2p = np.ascontiguousarray(Wfc2p, dtype=np.float32)

    in_maps = []
    for c in range(N_CORES):
        ilo, ihi, dlo, dhi = per_core[c]
        xs = np.zeros((NLP, P), dtype=np.float32)
        xs[:NL] = x[c * NL:(c + 1) * NL]
        x_t = np.ascontiguousarray(xs.T)                   # [128, NLP]

        degp = np.ones(NLP, dtype=np.float32)
        degp[:NL] = deg[c * NL:(c + 1) * NL]
        deg_t = np.ascontiguousarray(degp.reshape(NW, P).T)  # [128, NW]

        # dstrel: [128, NW*NCH] col = w*NCH + k, row p = edge slot
        drl = np.ascontiguousarray(
            dlo.reshape(NW, NCHL, P).transpose(2, 0, 1).reshape(P, NW * NCHL))
        drh = np.ascontiguousarray(
            dhi.reshape(NW, NCHH, P).transpose(2, 0, 1).reshape(P, NW * NCHH))

        hi_part = {}
        if NCHH:
            hi_part = {"idx_hi": _wrap_idx_groups(ihi, ngrp, GW),
                       "dstrel_hi": drh}
        in_maps.append({
            "x_t": x_t,
            "deg_t": deg_t,
            "idx_lo": _wrap_idx_groups(ilo, ngrp, GW),
            "dstrel_lo": drl,
            **hi_part,
            "iota": iota,
            "ident": ident,
            "Wg0": np.ascontiguousarray(Wg0, dtype=np.float32),
            "Wg1": np.ascontiguousarray(Wg1, dtype=np.float32),
            "Wg2": np.ascontiguousarray(Wg2, dtype=np.float32),
            "Wfc1": np.ascontiguousarray(Wfc1, dtype=np.float32),
            "Wfc2p": Wfc2p,
        })
    return in_maps


# --------------------------------------------------------------------------
# Device program
# --------------------------------------------------------------------------
def _build_bass(meta, mock_cc=False, opts=None, reps=1):
    opts = opts or {}
    from concourse import bass, bacc, mybir
    import concourse.tile as tile

    NW, NLP, NGP, HALF = meta["NW"], meta["NLP"], meta["NGP"], meta["HALF"]
    NCHL, NCHH = meta["NCHL"], meta["NCHH"]
    GW = _gw(NW)
    NGRP = NW // GW
    GLL = GW * NCHL * P                    # lo idxs per gather call
    GLH = GW * NCHH * P
    f32 = mybir.dt.float32
    bf16 = mybir.dt.bfloat16
    i16 = mybir.dt.int16
    ALL = [list(range(N_CORES))]

    nc = bacc.Bacc("TRN2", target_bir_lowering=False, debug=False,
                   num_devices=N_CORES)

    x_t_d = nc.dram_tensor("x_t", [P, NLP], f32, kind="ExternalInput")
    deg_d = nc.dram_tensor("deg_t", [P, NW], f32, kind="ExternalInput")
    ilo_d = nc.dram_tensor("idx_lo", [P, NW * NCHL * 8], i16, kind="ExternalInput")
    drl_d = nc.dram_tensor("dstrel_lo", [P, NW * NCHL], f32, kind="ExternalInput")
    if NCHH:
        ihi_d = nc.dram_tensor("idx_hi", [P, NW * NCHH * 8], i16,
                               kind="ExternalInput")
        drh_d = nc.dram_tensor("dstrel_hi", [P, NW * NCHH], f32,
                               kind="ExternalInput")
    iota_d = nc.dram_tensor("iota", [P, P], f32, kind="ExternalInput")
    ident_d = nc.dram_tensor("ident", [P, P], f32, kind="ExternalInput")
    wg_d = [nc.dram_tensor(f"Wg{i}", [P, P], f32, kind="ExternalInput")
            for i in range(3)]
    wfc1_d = nc.dram_tensor("Wfc1", [P, 256], f32, kind="ExternalInput")
    wfc2_d = nc.dram_tensor("Wfc2p", [P, 4], f32, kind="ExternalInput")
    out_d = nc.dram_tensor("out", [NLP, 2], f32, kind="ExternalOutput")

    with tile.TileContext(nc) as tc:
        with (
            tc.tile_pool(name="const", bufs=1) as cpool,
            tc.tile_pool(name="msg", bufs=2) as mpool,
            tc.tile_pool(name="oh", bufs=2) as ohpool,
            tc.tile_pool(name="work", bufs=3) as wpool,
            tc.tile_pool(name="acc", bufs=3, space="PSUM") as ppool,
            tc.tile_pool(name="accy", bufs=1, space="PSUM") as p2pool,
            tc.tile_pool(name="tpsum", bufs=4, space="PSUM") as tpool,
            tc.tile_pool(name="msgs", bufs=24) as mspool,
            tc.tile_pool(name="dram", bufs=1, space="DRAM") as dpool,
        ):
            # ---- constants / casts ----
            T_a = cpool.tile([P, NLP], bf16, name="T_a")
            nc.gpsimd.dma_start(out=T_a[:], in_=x_t_d[:])   # f32->bf16 cast
            T_b = cpool.tile([P, NLP], bf16, name="T_b")

            iota_sb = cpool.tile([P, P], bf16, name="iota_sb")
            nc.gpsimd.dma_start(out=iota_sb[:], in_=iota_d[:])
            ident_sb = cpool.tile([P, P], bf16, name="ident_sb")
            nc.gpsimd.dma_start(out=ident_sb[:], in_=ident_d[:])
            wg_sb = []
            for i in range(3):
                t = cpool.tile([P, P], bf16, name=f"wg_sb{i}")
                nc.gpsimd.dma_start(out=t[:], in_=wg_d[i][:])
                wg_sb.append(t)
            wfc1_sb = cpool.tile([P, 256], bf16, name="wfc1_sb")
            nc.gpsimd.dma_start(out=wfc1_sb[:], in_=wfc1_d[:])
            wfc2_sb = cpool.tile([P, 4], bf16, name="wfc2_sb")
            nc.gpsimd.dma_start(out=wfc2_sb[:], in_=wfc2_d[:])
            drl_sb = cpool.tile([P, NW * NCHL], bf16, name="drl_sb")
            nc.gpsimd.dma_start(out=drl_sb[:], in_=drl_d[:])  # f32->bf16
            ilo_sb = cpool.tile([P, NW * NCHL * 8], i16, name="ilo_sb")
            nc.sync.dma_start(out=ilo_sb[:], in_=ilo_d[:])
            if NCHH:
                drh_sb = cpool.tile([P, NW * NCHH], bf16, name="drh_sb")
                nc.gpsimd.dma_start(out=drh_sb[:], in_=drh_d[:])
                ihi_sb = cpool.tile([P, NW * NCHH * 8], i16, name="ihi_sb")
                nc.sync.dma_start(out=ihi_sb[:], in_=ihi_d[:])

            deg_sb = cpool.tile([P, NW], f32, name="deg_sb")
            nc.sync.dma_start(out=deg_sb[:], in_=deg_d[:])
            invdeg = cpool.tile([P, NW], f32, name="invdeg")
            nc.vector.reciprocal(invdeg[:], deg_sb[:])
            dinv = cpool.tile([P, NW], f32, name="dinv")
            nc.scalar.sqrt(dinv[:], invdeg[:])

            # NB: collective outputs in Local addr space — Shared
            # scratchpad DMA reads measured ~3x slower on the gather path.
            gfull = [dpool.tile([NGP, P], bf16, name=f"gfull{i}")
                     for i in range(3)]
            gloc = [dpool.tile([NLP, P], bf16, name=f"gloc{i}")
                    for i in range(3)]

            Copy = mybir.ActivationFunctionType.Copy

            def g_production(l, Tsrc):
                scale = dinv if l == 0 else invdeg
                for w in range(NW):
                    ps = ppool.tile([P, P], f32, tag="acc", name="psg")
                    nc.tensor.matmul(ps[:], lhsT=Tsrc[:, w * P:(w + 1) * P],
                                     rhs=wg_sb[l][:], start=True, stop=True)
                    gw_t = wpool.tile([P, P], bf16, tag="gw", name="gw_t")
                    nc.scalar.activation(gw_t[:], ps[:], Copy,
                                         bias=0.0, scale=scale[:, w:w + 1])
                    nc.sync.dma_start(out=gloc[l][w * P:(w + 1) * P, :],
                                      in_=gw_t[:])
                if mock_cc:
                    # single-core timing mock: local copy approximating the
                    # AllGather's local write volume
                    for c in range(N_CORES):
                        nc.sync.dma_start(
                            out=gfull[l][c * NLP:(c + 1) * NLP, :],
                            in_=gloc[l][:])
                else:
                    nc.gpsimd.collective_compute(
                        "AllGather", mybir.AluOpType.bypass,
                        replica_groups=ALL,
                        ins=[gloc[l][:]], outs=[gfull[l][:]])

            def leaky_into(dst_ap, ps):
                t = wpool.tile([P, dst_ap.shape[-1]], f32, tag="lk", name="lkt")
                nc.scalar.activation(t[:], ps[:], Copy, bias=0.0, scale=0.01)
                nc.vector.tensor_tensor(out=dst_ap, in0=ps[:], in1=t[:],
                                        op=mybir.AluOpType.max)


            ohb_const = {}
            if opts.get("skip_ohbuild"):
                GWl = _gw(NW)
                KL = GWl * NCHL
                t = cpool.tile([P, KL * P], bf16, name="ohc_lo")
                nc.vector.tensor_tensor(
                    out=t[:].rearrange("p (k r) -> p k r", r=P),
                    in0=iota_sb[:].unsqueeze(1).to_broadcast([P, KL, P]),
                    in1=drl_sb[:, 0:KL].unsqueeze(2)
                        .to_broadcast([P, KL, P]),
                    op=mybir.AluOpType.is_equal)
                ohb_const["lo"] = t
                if NCHH:
                    KH = GWl * NCHH
                    t2 = cpool.tile([P, KH * P], bf16, name="ohc_hi")
                    nc.vector.tensor_tensor(
                        out=t2[:].rearrange("p (k r) -> p k r", r=P),
                        in0=iota_sb[:].unsqueeze(1).to_broadcast([P, KH, P]),
                        in1=drh_sb[:, 0:KH].unsqueeze(2)
                            .to_broadcast([P, KH, P]),
                        op=mybir.AluOpType.is_equal)
                    ohb_const["hi"] = t2

            def scatter_tr(l, Tdst):
                """Transpose-mode gather variant: dma_gather(transpose=True)
                returns messages feature-major [128f, n_idxs]; each 128-edge
                chunk is PE-transposed back to edge-major via the identity
                trick, staged through PSUM(bf16) -> SBUF, then accumulated
                with the one-hot matmul as before."""
                glo_ap = gfull[l][0:min(HALF, NGP), :]
                ghi_ap = gfull[l][HALF:NGP, :] if NCHH else None
                ntot = NCHL + NCHH
                for g in range(NGRP):
                    mlo = mpool.tile([P, GLL], bf16, tag="mlo", name="mlo")
                    nc.gpsimd.dma_gather(
                        out_ap=mlo[:].rearrange("p (c e) -> p c e", c=1),
                        in_ap=glo_ap,
                        idxs_ap=ilo_sb[:, g * (GLL // 16):
                                       (g + 1) * (GLL // 16)],
                        num_idxs=GLL, num_idxs_reg=GLL, elem_size=P,
                        transpose=True, single_packet=False)
                    if NCHH:
                        mhi = mpool.tile([P, GLH], bf16, tag="mhi", name="mhi")
                        nc.gpsimd.dma_gather(
                            out_ap=mhi[:].rearrange("p (c e) -> p c e", c=1),
                            in_ap=ghi_ap,
                            idxs_ap=ihi_sb[:, g * (GLH // 16):
                                           (g + 1) * (GLH // 16)],
                            num_idxs=GLH, num_idxs_reg=GLH, elem_size=P,
                            transpose=True, single_packet=False)
                    KL = GW * NCHL
                    ohb_lo = ohpool.tile([P, KL * P], bf16, tag="ohlo",
                                         name="ohb_lo")
                    nc.vector.tensor_tensor(
                        out=ohb_lo[:].rearrange("p (k r) -> p k r", r=P),
                        in0=iota_sb[:].unsqueeze(1)
                            .to_broadcast([P, KL, P]),
                        in1=drl_sb[:, g * KL:(g + 1) * KL].unsqueeze(2)
                            .to_broadcast([P, KL, P]),
                        op=mybir.AluOpType.is_equal)
                    if NCHH:
                        KH = GW * NCHH
                        ohb_hi = ohpool.tile([P, KH * P], bf16,
                                             tag="ohhi", name="ohb_hi")
                        nc.vector.tensor_tensor(
                            out=ohb_hi[:].rearrange("p (k r) -> p k r",
                                                    r=P),
                            in0=iota_sb[:].unsqueeze(1)
                                .to_broadcast([P, KH, P]),
                            in1=drh_sb[:, g * KH:(g + 1) * KH]
                                .unsqueeze(2).to_broadcast([P, KH, P]),
                            op=mybir.AluOpType.is_equal)
                    for wi in range(GW):
                        w = g * GW + wi

                        def chunk_aps(k):
                            if k < NCHL:
                                c = wi * NCHL + k
                                return (mlo[:, c * P:(c + 1) * P],
                                        ohb_lo[:, c * P:(c + 1) * P])
                            c = wi * NCHH + (k - NCHL)
                            return (mhi[:, c * P:(c + 1) * P],
                                    ohb_hi[:, c * P:(c + 1) * P])

                        # batch all transposes + PSUM->SBUF copies ahead of
                        # the accumulate chain so mm2 never stalls on the
                        # PSUM->ACT->SBUF round trip
                        msgs = []
                        for k in range(ntot):
                            mT_ap, _ = chunk_aps(k)
                            psT = tpool.tile([P, P], bf16, tag="tr",
                                             name="psT")
                            nc.tensor.transpose(psT[:], mT_ap, ident_sb[:])
                            msg = mspool.tile([P, P], bf16, tag="msg",
                                              name="msg")
                            nc.scalar.activation(msg[:], psT[:], Copy,
                                                 bias=0.0, scale=1.0)
                            msgs.append(msg)
                        ps = ppool.tile([P, P], f32, tag="acc", name="pss")
                        for k in range(ntot):
                            _, oh_ap = chunk_aps(k)
                            nc.tensor.matmul(ps[:], lhsT=msgs[k][:],
                                             rhs=oh_ap,
                                             start=(k == 0),
                                             stop=(k == ntot - 1))
                        leaky_into(Tdst[:, w * P:(w + 1) * P], ps)

            def scatter(l, Tdst):
                glo_ap = gfull[l][0:min(HALF, NGP), :]
                ghi_ap = gfull[l][HALF:NGP, :] if NCHH else None
                ntot = NCHL + NCHH
                for g in range(NGRP):
                    mlo = mpool.tile([P, GLL], bf16, tag="mlo", name="mlo")
                    if opts.get("fake_gather"):
                        nc.sync.dma_start(
                            out=mlo[:].rearrange("p (c e) -> p c e", e=P),
                            in_=gfull[l][0:GLL, :].rearrange(
                                "(c p) e -> p c e", p=P))
                    else:
                        nc.gpsimd.dma_gather(
                            out_ap=mlo[:].rearrange("p (c e) -> p c e", e=P),
                            in_ap=glo_ap,
                            idxs_ap=ilo_sb[:, g * (GLL // 16):
                                           (g + 1) * (GLL // 16)],
                            num_idxs=GLL, num_idxs_reg=GLL, elem_size=P,
                            single_packet=False)
                    if NCHH:
                        mhi = mpool.tile([P, GLH], bf16, tag="mhi", name="mhi")
                        if opts.get("fake_gather"):
                            nc.sync.dma_start(
                                out=mhi[:].rearrange("p (c e) -> p c e", e=P),
                                in_=gfull[l][0:GLH, :].rearrange(
                                    "(c p) e -> p c e", p=P))
                        else:
                            nc.gpsimd.dma_gather(
                                out_ap=mhi[:].rearrange("p (c e) -> p c e",
                                                        e=P),
                                in_ap=ghi_ap,
                                idxs_ap=ihi_sb[:, g * (GLH // 16):
                                               (g + 1) * (GLH // 16)],
                                num_idxs=GLH, num_idxs_reg=GLH, elem_size=P,
                                single_packet=False)
                    # batched one-hot build: one DVE op per (group, half)
                    KL = GW * NCHL
                    if opts.get("skip_ohbuild"):
                        ohb_lo = ohb_const["lo"]
                    else:
                        ohb_lo = ohpool.tile([P, KL * P], bf16, tag="ohlo",
                                             name="ohb_lo")
                        nc.vector.tensor_tensor(
                            out=ohb_lo[:].rearrange("p (k r) -> p k r", r=P),
                            in0=iota_sb[:].unsqueeze(1)
                                .to_broadcast([P, KL, P]),
                            in1=drl_sb[:, g * KL:(g + 1) * KL].unsqueeze(2)
                                .to_broadcast([P, KL, P]),
                            op=mybir.AluOpType.is_equal)
                    if NCHH:
                        KH = GW * NCHH
                        if opts.get("skip_ohbuild"):
                            ohb_hi = ohb_const["hi"]
                        else:
                            ohb_hi = ohpool.tile([P, KH * P], bf16,
                                                 tag="ohhi", name="ohb_hi",
                                                 bufs=opts.get("hibufs", 1))
                            nc.vector.tensor_tensor(
                                out=ohb_hi[:].rearrange("p (k r) -> p k r",
                                                        r=P),
                                in0=iota_sb[:].unsqueeze(1)
                                    .to_broadcast([P, KH, P]),
                                in1=drh_sb[:, g * KH:(g + 1) * KH]
                                    .unsqueeze(2).to_broadcast([P, KH, P]),
                                op=mybir.AluOpType.is_equal)
                    for wi in range(GW):
                        w = g * GW + wi
                        ps = ppool.tile([P, P], f32, tag="acc", name="pss")
                        if not opts.get("skip_matmul"):
                            for k in range(ntot):
                                if k < NCHL:
                                    c = wi * NCHL + k
                                    m_ap = mlo[:, c * P:(c + 1) * P]
                                    oh_ap = ohb_lo[:, c * P:(c + 1) * P]
                                else:
                                    c = wi * NCHH + (k - NCHL)
                                    m_ap = mhi[:, c * P:(c + 1) * P]
                                    oh_ap = ohb_hi[:, c * P:(c + 1) * P]
                                nc.tensor.matmul(ps[:], lhsT=m_ap, rhs=oh_ap,
                                                 start=(k == 0),
                                                 stop=(k == ntot - 1))
                            if not opts.get("skip_leaky"):
                                leaky_into(Tdst[:, w * P:(w + 1) * P], ps)

            def head(Tsrc):
                X = mybir.AxisListType.X
                Exp = mybir.ActivationFunctionType.Exp
                for w in range(NW):
                    y1t = []
                    for h in range(2):
                        ps1 = ppool.tile([P, P], f32, tag="acc", name="ps1")
                        nc.tensor.matmul(ps1[:],
                                         lhsT=wfc1_sb[:, h * P:(h + 1) * P],
                                         rhs=Tsrc[:, w * P:(w + 1) * P],
                                         start=True, stop=True)
                        yt = wpool.tile([P, P], bf16, tag=f"y1_{h}",
                                        name="yt")
                        leaky_into(yt[:], ps1)
                        y1t.append(yt)
                    ps2 = p2pool.tile([P, 2], f32, tag="y2", name="ps2")
                    nc.tensor.matmul(ps2[:], lhsT=y1t[0][:],
                                     rhs=wfc2_sb[:, 0:2],
                                     start=True, stop=False)
                    nc.tensor.matmul(ps2[:], lhsT=y1t[1][:],
                                     rhs=wfc2_sb[:, 2:4],
                                     start=False, stop=True)
                    y2 = wpool.tile([P, 2], f32, tag="y2s", name="y2")
                    leaky_into(y2[:], ps2)
                    z = wpool.tile([P, 2], f32, tag="z", name="z")
                    nc.scalar.activation(z[:], y2[:], Copy, bias=0.0,
                                         scale=dinv[:, w:w + 1])
                    negm = wpool.tile([P, 1], f32, tag="m", name="negm")
                    nc.vector.reduce_max(out=negm[:], in_=z[:], axis=X,
                                         negate=True)
                    e = wpool.tile([P, 2], f32, tag="e", name="e")
                    nc.scalar.activation(e[:], z[:], Exp,
                                         bias=negm[:, 0:1], scale=1.0)
                    s = wpool.tile([P, 1], f32, tag="s", name="s")
                    nc.vector.reduce_sum(out=s[:], in_=e[:], axis=X)
                    rs = wpool.tile([P, 1], f32, tag="rs", name="rs")
                    nc.vector.reciprocal(rs[:], s[:])
                    o = wpool.tile([P, 2], f32, tag="o", name="o")
                    nc.vector.tensor_scalar(out=o[:], in0=e[:],
                                            scalar1=rs[:, 0:1], scalar2=None,
                                            op0=mybir.AluOpType.mult)
                    nc.sync.dma_start(out=out_d[w * P:(w + 1) * P, :],
                                      in_=o[:])

            sc = scatter if opts.get("classic") else scatter_tr
            for _rep in range(reps):
                if opts.get("skip_scatter"):
                    g_production(0, T_a)
                    g_production(1, T_a)
                    g_production(2, T_a)
                    head(T_a)
                else:
                    g_production(0, T_a)
                    sc(0, T_b)
                    g_production(1, T_b)
                    sc(1, T_a)
                    g_production(2, T_a)
                    sc(2, T_b)
                    head(T_b)

    nc.compile()
    return nc


# --------------------------------------------------------------------------
# Entry point
# --------------------------------------------------------------------------
LAST_RESULT = None
LAST_NC = None
LAST_IN_MAPS = None
LAST_META = None


def kernel(x, edge_index, Wg0, Wg1, Wg2, Wfc1, Wfc2):
    from concourse.bass_utils import run_bass_kernel_spmd

    global LAST_RESULT, LAST_NC, LAST_IN_MAPS, LAST_META
    x = np.asarray(x)
    edge_index = np.asarray(edge_index)
    per_core, meta = _preprocess(x, edge_index)
    in_maps = _build_core_inputs(
        x, (np.asarray(Wg0), np.asarray(Wg1), np.asarray(Wg2),
            np.asarray(Wfc1), np.asarray(Wfc2)), per_core, meta)
    nc = _build_bass(meta)
    LAST_NC, LAST_IN_MAPS, LAST_META = nc, in_maps, meta
    res = run_bass_kernel_spmd(nc, in_maps, core_ids=list(range(N_CORES)))
    LAST_RESULT = res
    NL = meta["NL"]
    out = np.concatenate([res.results[c]["out"][:NL] for c in range(N_CORES)],
                         axis=0)
    return out.astype(np.float32)

